# revision 11
# baseline (speedup 1.0000x reference)
"""DFT Multi-Head Attention kernel for 8x Trainium2 NeuronCores.

Math: by Parseval's theorem, for real q,k:
    sum_f FFT(q)[f] * conj(FFT(k)[f]) = D * (q . k)      (exactly real)
so |scores| = D * |q . k| and the whole DFT attention collapses to
    out = softmax(D * |Q K^T|) V
i.e. standard attention with an elementwise abs and a scale of D=64 on the
logits.  The softmax over these logits is extremely peaked (logit scale
~512), so the q/k path must be computed at fp32-grade precision: bf16
matmuls are emulated-fp32 via 3-term Dekker splits (hi*hi + lo*hi + hi*lo).
The p/v path (post-softmax) runs in plain bf16.

Sharding (8 cores): core c handles batch b=c//4 and heads 4*(c%4)..4*(c%4)+3
(data + head parallel).  Q/K/V projection weights are column-sharded and the
out-projection weights row-sharded (Megatron style); each core computes a
partial [S, E] output and the host sums 4 partials per batch and adds bo.

Per-core pipeline (all on-chip):
  1. q^T,k^T projections into head-major pair layout [128=2x64dims, S],
     split into bf16 hi/lo; v projection into s-major [S, 4*64] bf16.
  2. Per (head-pair, 128-row q-block): 3-term QK^T matmuls, two heads packed
     into the 128x128 PE array via row tile_position (K=64 each);
     fused DVE pass (tensor_tensor_reduce) computes amp=|s| and row-max
     straight out of PSUM; ScalarE exp(amp-m) with accumulated row-sums;
     row-normalize; PE transpose-mode to get p^T; PV matmuls with the two
     heads packed via column tile_position.
  3. Out-projection of the concatenated head outputs (row-sharded WoT).
"""

import sys

sys.path.insert(0, "/opt/trn_rl_repo")

from contextlib import ExitStack

import ml_dtypes
import numpy as np

import concourse.bass as bass
import concourse.mybir as mybir
import concourse.tile as tile
from concourse import bacc
from concourse.bass_utils import run_bass_kernel_spmd
from concourse.masks import make_identity

BF16 = ml_dtypes.bfloat16
F32 = mybir.dt.float32
B16 = mybir.dt.bfloat16

# problem dims (fixed by the task)
B, S, E, H = 2, 2048, 1024, 16
D = 64
N_CORES = 8
HPC = 4  # heads per core
DHC = HPC * D  # head dims per core = 256


def build_program(s=S, e_in=E, e_out=E):
    """Emit the per-core SPMD program. Returns the compiled Bacc."""
    assert s % 512 == 0 and e_in % 128 == 0 and e_out % 512 == 0
    EC = e_in // 128  # contraction chunks for projections
    NC = s // 512  # 512-wide column chunks of S
    QB = s // 128  # 128-row q blocks
    KT = s // 128  # 128-wide k tiles (transposes / PV contraction)
    E2 = e_out // 512

    nc = bacc.Bacc("TRN2", target_bir_lowering=False, debug=False)

    def din(name, shape, dt=B16):
        return nc.dram_tensor(name, shape, dt, kind="ExternalInput").ap()

    qhiT_d = din("qhiT", [e_in, s])
    qloT_d = din("qloT", [e_in, s])
    khiT_d = din("khiT", [e_in, s])
    kloT_d = din("kloT", [e_in, s])
    vT_d = din("vT", [e_in, s])
    wqhi_d = din("wqhi", [e_in, DHC])
    wqlo_d = din("wqlo", [e_in, DHC])
    wkhi_d = din("wkhi", [e_in, DHC])
    wklo_d = din("wklo", [e_in, DHC])
    wv_d = din("wv", [e_in, DHC])
    wo_d = din("wo", [DHC, e_out])
    bq_d = din("bq", [1, DHC])
    bk_d = din("bk", [1, DHC])
    bv_d = din("bv", [1, DHC])
    out_d = nc.dram_tensor("out", [s, e_out], F32, kind="ExternalOutput").ap()

    with tile.TileContext(nc) as tc, ExitStack() as ctx:
        # ---- constants & resident weights ----
        const = ctx.enter_context(tc.tile_pool(name="const", bufs=1))
        ident = const.tile([128, 128], B16)
        make_identity(nc, ident[:])
        ones512 = const.tile([1, 512], B16)
        nc.vector.memset(ones512[:], 1.0)
        ones128 = const.tile([1, 128], B16)
        nc.vector.memset(ones128[:], 1.0)

        wpool = ctx.enter_context(tc.tile_pool(name="weights", bufs=1))
        wt = {}
        for nm, dram in [
            ("wqhi", wqhi_d),
            ("wqlo", wqlo_d),
            ("wkhi", wkhi_d),
            ("wklo", wklo_d),
            ("wv", wv_d),
        ]:
            for ei in range(EC):
                t = wpool.tile([128, DHC], B16, tag=f"{nm}{ei}", name=f"{nm}{ei}")
                nc.sync.dma_start(t[:], dram[ei * 128 : (ei + 1) * 128, :])
                wt[nm, ei] = t
        wo_t = []
        for pair in range(2):
            t = wpool.tile([128, e_out], B16, tag=f"wo{pair}", name=f"wo{pair}")
            nc.sync.dma_start(t[:], wo_d[pair * 128 : (pair + 1) * 128, :])
            wo_t.append(t)
        brow = {}
        for nm, dram in [("bq", bq_d), ("bk", bk_d), ("bv", bv_d)]:
            t = wpool.tile([1, DHC], B16, tag=nm, name=f"b{nm}")
            nc.sync.dma_start(t[:], dram[:])
            brow[nm] = t

        # ---- persistent activations ----
        act = ctx.enter_context(tc.tile_pool(name="acts", bufs=1))
        # q/k in pair layout: [128 = headA(64) | headB(64), s], bf16 hi/lo
        qk = {}
        for nm in ("qhi", "qlo", "khi", "klo"):
            for pair in range(2):
                qk[nm, pair] = act.tile([128, s], B16, tag=f"{nm}p{pair}", name=f"{nm}p{pair}")
        v_all = act.tile([128, KT * DHC], B16)  # per k-tile: 4 heads x 64
        outT = [act.tile([128, s], B16, tag=f"outT{p}", name=f"outT{p}") for p in range(2)]

        # ================= Phase 1: projections =================
        with (
            tc.tile_pool(name="xin", bufs=4) as xin,
            tc.tile_pool(name="psproj", bufs=2, space="PSUM") as psproj,
        ):
            for src_hi, src_lo, whi, wlo, bias, dhi, dlo in (
                (qhiT_d, qloT_d, "wqhi", "wqlo", "bq", "qhi", "qlo"),
                (khiT_d, kloT_d, "wkhi", "wklo", "bk", "khi", "klo"),
            ):
                for n in range(NC):
                    xh, xl = [], []
                    for ei in range(EC):
                        th = xin.tile([128, 512], B16, tag="xh", bufs=EC + 2)
                        nc.sync.dma_start(
                            th[:], src_hi[ei * 128 : (ei + 1) * 128, n * 512 : (n + 1) * 512]
                        )
                        xh.append(th)
                        tl = xin.tile([128, 512], B16, tag="xl", bufs=EC + 2)
                        nc.sync.dma_start(
                            tl[:], src_lo[ei * 128 : (ei + 1) * 128, n * 512 : (n + 1) * 512]
                        )
                        xl.append(tl)
                    for pair in range(2):
                        ps = psproj.tile([128, 512], F32)
                        terms = [(whi, xh), (wlo, xh), (whi, xl)]
                        for t_i, (wn, xs) in enumerate(terms):
                            for ei in range(EC):
                                nc.tensor.matmul(
                                    ps[:],
                                    wt[wn, ei][:, pair * 128 : (pair + 1) * 128],
                                    xs[ei][:],
                                    start=(t_i == 0 and ei == 0),
                                    stop=False,
                                )
                        # bias: out[dg, col] += bias[dg] * 1
                        nc.tensor.matmul(
                            ps[:],
                            brow[bias][0:1, pair * 128 : (pair + 1) * 128],
                            ones512[0:1, :],
                            start=False,
                            stop=True,
                        )
                        hi_dst = qk[dhi, pair][:, n * 512 : (n + 1) * 512]
                        lo_dst = qk[dlo, pair][:, n * 512 : (n + 1) * 512]
                        nc.scalar.copy(hi_dst, ps[:])
                        nc.vector.tensor_sub(lo_dst, ps[:], hi_dst)

            # v projection: v[s_chunk, 4 heads * 64] accumulated over e
            with tc.tile_pool(name="psv", bufs=2, space="PSUM") as psv:
                for sc in range(KT):
                    ps = psv.tile([128, DHC], F32)
                    for ei in range(EC):
                        tv = xin.tile([128, 128], B16, tag="xv")
                        nc.sync.dma_start(
                            tv[:], vT_d[ei * 128 : (ei + 1) * 128, sc * 128 : (sc + 1) * 128]
                        )
                        nc.tensor.matmul(
                            ps[:], tv[:], wt["wv", ei][:], start=(ei == 0), stop=False
                        )
                    nc.tensor.matmul(
                        ps[:], ones128[0:1, :], brow["bv"][0:1, :], start=False, stop=True
                    )
                    nc.scalar.copy(v_all[:, sc * DHC : (sc + 1) * DHC], ps[:])

        # ================= Phase 2: attention =================
        with (
            tc.tile_pool(name="amp", bufs=2) as amppool,
            tc.tile_pool(name="pp", bufs=2) as ppool,
            tc.tile_pool(name="pt", bufs=1) as ptpool,
            tc.tile_pool(name="small", bufs=8) as small,
            tc.tile_pool(name="pss", bufs=2, space="PSUM") as pss,
            tc.tile_pool(name="pstr", bufs=1, space="PSUM") as pstr,
            tc.tile_pool(name="pspv", bufs=1, space="PSUM") as pspv,
        ):
            for pair in range(2):
                # p^T accumulator for one q-group (4 q-blocks): [128, kt, 4, 128]
                pts = {}
                for qb in range(QB):
                    qsl = slice(qb * 128, (qb + 1) * 128)
                    if qb % 4 == 0:
                        pts[0] = ptpool.tile([128, KT, 4, 128], B16, tag="ptA", name="ptA")
                        pts[1] = ptpool.tile([128, KT, 4, 128], B16, tag="ptB", name="ptB")
                    amp = [
                        amppool.tile([128, s], F32, tag="ampA", name="ampA"),
                        amppool.tile([128, s], F32, tag="ampB", name="ampB"),
                    ]
                    m = [small.tile([128, 1], F32, tag=f"m{h}", name=f"m{h}") for h in range(2)]
                    for c in range(NC):
                        csl = slice(c * 512, (c + 1) * 512)
                        sps = [pss.tile([128, 512], F32, tag=f"s{h}", name=f"sps{h}") for h in range(2)]
                        terms = (("qhi", "khi"), ("qlo", "khi"), ("qhi", "klo"))
                        for h, (r0, r1, tp) in enumerate(
                            ((0, 64, (0, 0)), (64, 128, (64, 0)))
                        ):
                            for t_i, (qn, kn) in enumerate(terms):
                                nc.tensor.matmul(
                                    sps[h][:],
                                    qk[qn, pair][r0:r1, qsl],
                                    qk[kn, pair][r0:r1, csl],
                                    tile_position=tp,
                                    start=(t_i == 0),
                                    stop=(t_i == 2),
                                )
                        for h in range(2):
                            # amp = |s|, evacuating PSUM promptly (ScalarE)
                            nc.scalar.activation(
                                amp[h][:, csl],
                                sps[h][:],
                                mybir.ActivationFunctionType.Abs,
                            )
                    for h in range(2):
                        nc.vector.tensor_reduce(
                            m[h][:],
                            amp[h][:],
                            axis=mybir.AxisListType.X,
                            op=mybir.AluOpType.max,
                        )
                        negm = small.tile([128, 1], F32, tag=f"negm{h}", name=f"negm{h}")
                        nc.vector.tensor_scalar_mul(negm[:], m[h][:], -1.0)
                        p = ppool.tile([128, s], B16, tag=f"p{h}", name=f"p{h}")
                        sig = small.tile([128, 1], F32, tag=f"sig{h}", name=f"sig{h}")
                        nc.scalar.activation(
                            p[:],
                            amp[h][:],
                            mybir.ActivationFunctionType.Exp,
                            bias=negm[:, 0:1],
                            scale=1.0,
                            accum_out=sig[:, 0:1],
                        )
                        r = small.tile([128, 1], F32, tag=f"r{h}", name=f"r{h}")
                        nc.vector.reciprocal(r[:], sig[:])
                        nc.gpsimd.tensor_scalar_mul(p[:], p[:], r[:, 0:1])
                        pn = p
                        # transpose 128x128 tiles: p^T into PSUM (bf16), TB per bank
                        TB = min(8, KT)
                        for half in range(KT // TB):
                            trp = pstr.tile([128, TB, 128], B16, tag=f"tr{h}", name=f"tr{h}")
                            for j in range(TB):
                                kc = half * TB + j
                                nc.tensor.transpose(
                                    trp[:, j, :],
                                    pn[:, kc * 128 : (kc + 1) * 128],
                                    ident[:],
                                )
                            pt_dst = pts[h][:, half * TB : (half + 1) * TB, qb % 4, :]
                            if half % 2 == 0:
                                nc.vector.tensor_copy(pt_dst, trp[:])
                            else:
                                nc.scalar.copy(pt_dst, trp[:])
                    if qb % 4 == 3:
                        qg = qb // 4
                        pso = pspv.tile([128, 512], F32)
                        for kc in range(KT):
                            for h, tp in ((0, (0, 0)), (1, (0, 64))):
                                hc = pair * 2 + h
                                nc.tensor.matmul(
                                    pso[h * 64 : (h + 1) * 64, :],
                                    v_all[:, kc * DHC + hc * 64 : kc * DHC + (hc + 1) * 64],
                                    pts[h][:, kc, :, :],
                                    tile_position=tp,
                                    start=(kc == 0),
                                    stop=(kc == KT - 1),
                                    skip_group_check=True,
                                )
                        nc.scalar.copy(outT[pair][:, qg * 512 : (qg + 1) * 512], pso[:])

        # ================= Phase 3: out-projection =================
        with (
            tc.tile_pool(name="osb", bufs=2) as osb,
            tc.tile_pool(name="psf", bufs=2, space="PSUM") as psf,
        ):
            for sc in range(KT):
                osl = slice(sc * 128, (sc + 1) * 128)
                ot = osb.tile([128, e_out], F32)
                for e2 in range(E2):
                    ps = psf.tile([128, 512], F32)
                    for pair in range(2):
                        nc.tensor.matmul(
                            ps[:],
                            outT[pair][:, osl],
                            wo_t[pair][:, e2 * 512 : (e2 + 1) * 512],
                            start=(pair == 0),
                            stop=(pair == 1),
                        )
                    nc.scalar.copy(ot[:, e2 * 512 : (e2 + 1) * 512], ps[:])
                nc.sync.dma_start(out_d[osl, :], ot[:])

    nc.compile()
    return nc


_PROG = None


def _get_program():
    global _PROG
    if _PROG is None:
        _PROG = build_program()
    return _PROG


def _split(x):
    hi = x.astype(BF16)
    lo = (x - hi.astype(np.float32)).astype(BF16)
    return hi, lo


def make_in_maps(query, key_, value, Wq, bq, Wk, bk, Wv, bv, Wo):
    per_batch = []
    for b in range(B):
        qT = np.ascontiguousarray(query[b].T.astype(np.float32))
        kT = np.ascontiguousarray(key_[b].T.astype(np.float32))
        vT = np.ascontiguousarray(value[b].T.astype(np.float32)).astype(BF16)
        qhi, qlo = _split(qT)
        khi, klo = _split(kT)
        per_batch.append((qhi, qlo, khi, klo, vT))
    WqT = np.ascontiguousarray((64.0 * Wq).T.astype(np.float32))
    WkT = np.ascontiguousarray(Wk.T.astype(np.float32))
    WvT = np.ascontiguousarray(Wv.T.astype(np.float32))
    WoT = np.ascontiguousarray(Wo.T.astype(np.float32))
    in_maps = []
    for c in range(N_CORES):
        b, g = c // 4, c % 4
        cols = slice(g * DHC, (g + 1) * DHC)
        qhi, qlo, khi, klo, vT = per_batch[b]
        wqhi, wqlo = _split(np.ascontiguousarray(WqT[:, cols]))
        wkhi, wklo = _split(np.ascontiguousarray(WkT[:, cols]))
        in_maps.append(
            {
                "qhiT": qhi,
                "qloT": qlo,
                "khiT": khi,
                "kloT": klo,
                "vT": vT,
                "wqhi": wqhi,
                "wqlo": wqlo,
                "wkhi": wkhi,
                "wklo": wklo,
                "wv": np.ascontiguousarray(WvT[:, cols]).astype(BF16),
                "wo": np.ascontiguousarray(WoT[cols, :]).astype(BF16),
                "bq": (64.0 * bq[cols]).reshape(1, DHC).astype(BF16),
                "bk": bk[cols].reshape(1, DHC).astype(BF16),
                "bv": bv[cols].reshape(1, DHC).astype(BF16),
            }
        )
    return in_maps


TRACE = False


def kernel(query, key_, value, Wq, bq, Wk, bk, Wv, bv, Wo, bo, _res_out=None):
    nc = _get_program()
    in_maps = make_in_maps(query, key_, value, Wq, bq, Wk, bk, Wv, bv, Wo)
    res = run_bass_kernel_spmd(nc, in_maps, list(range(N_CORES)), trace=TRACE)
    out = np.zeros((B, S, E), np.float32)
    for c in range(N_CORES):
        out[c // 4] += res.results[c]["out"]
    out += bo.astype(np.float32)
    if _res_out is not None:
        _res_out.append(res)
    return out


# revision 12
# speedup vs baseline: 1.0965x; 1.0965x over previous
"""DFT Multi-Head Attention kernel for 8x Trainium2 NeuronCores.

Math: by Parseval's theorem, for real q,k:
    sum_f FFT(q)[f] * conj(FFT(k)[f]) = D * (q . k)      (exactly real)
so |scores| = D * |q . k| and the whole DFT attention collapses to
    out = softmax(D * |Q K^T|) V
i.e. standard attention with an elementwise abs and a scale of D=64 on the
logits.  The softmax over these logits is extremely peaked (logit scale
~512), so the q/k path must be computed at fp32-grade precision: bf16
matmuls are emulated-fp32 via 3-term Dekker splits (hi*hi + lo*hi + hi*lo).
The p/v path (post-softmax) runs in plain bf16.

Sharding (8 cores): core c handles batch b=c//4 and heads 4*(c%4)..4*(c%4)+3
(data + head parallel).  Q/K/V projection weights are column-sharded and the
out-projection weights row-sharded (Megatron style); each core computes a
partial [S, E] output and the host sums 4 partials per batch and adds bo.

Per-core pipeline (all on-chip):
  1. q^T,k^T projections into head-major pair layout [128=2x64dims, S],
     split into bf16 hi/lo; v projection into s-major [S, 4*64] bf16.
  2. Per (head-pair, 128-row q-block): 3-term QK^T matmuls, two heads packed
     into the 128x128 PE array via row tile_position (K=64 each);
     fused DVE pass (tensor_tensor_reduce) computes amp=|s| and row-max
     straight out of PSUM; ScalarE exp(amp-m) with accumulated row-sums;
     row-normalize; PE transpose-mode to get p^T; PV matmuls with the two
     heads packed via column tile_position.
  3. Out-projection of the concatenated head outputs (row-sharded WoT).
"""

import sys

sys.path.insert(0, "/opt/trn_rl_repo")

from contextlib import ExitStack

import ml_dtypes
import numpy as np

import concourse.bass as bass
import concourse.mybir as mybir
import concourse.tile as tile
from concourse import bacc
from concourse.bass_utils import run_bass_kernel_spmd
from concourse.masks import make_identity

BF16 = ml_dtypes.bfloat16
F32 = mybir.dt.float32
B16 = mybir.dt.bfloat16

# problem dims (fixed by the task)
B, S, E, H = 2, 2048, 1024, 16
D = 64
N_CORES = 8
HPC = 4  # heads per core
DHC = HPC * D  # head dims per core = 256


def build_program(s=S, e_in=E, e_out=E):
    """Emit the per-core SPMD program. Returns the compiled Bacc."""
    assert s % 512 == 0 and e_in % 128 == 0 and e_out % 512 == 0
    EC = e_in // 128  # contraction chunks for projections
    NC = s // 512  # 512-wide column chunks of S
    QB = s // 128  # 128-row q blocks
    KT = s // 128  # 128-wide k tiles (transposes / PV contraction)
    E2 = e_out // 512

    nc = bacc.Bacc("TRN2", target_bir_lowering=False, debug=False)

    def din(name, shape, dt=B16):
        return nc.dram_tensor(name, shape, dt, kind="ExternalInput").ap()

    qhiT_d = din("qhiT", [e_in, s])
    qloT_d = din("qloT", [e_in, s])
    khiT_d = din("khiT", [e_in, s])
    kloT_d = din("kloT", [e_in, s])
    vT_d = din("vT", [e_in, s])
    wqhi_d = din("wqhi", [e_in, DHC])
    wqlo_d = din("wqlo", [e_in, DHC])
    wkhi_d = din("wkhi", [e_in, DHC])
    wklo_d = din("wklo", [e_in, DHC])
    wv_d = din("wv", [e_in, DHC])
    wo_d = din("wo", [DHC, e_out])
    bq_d = din("bq", [1, DHC])
    bk_d = din("bk", [1, DHC])
    bv_d = din("bv", [1, DHC])
    out_d = nc.dram_tensor("out", [s, e_out], F32, kind="ExternalOutput").ap()

    with tile.TileContext(nc) as tc, ExitStack() as ctx:
        # ---- constants & resident weights ----
        const = ctx.enter_context(tc.tile_pool(name="const", bufs=1))
        ident = const.tile([128, 128], B16)
        make_identity(nc, ident[:])
        ones512 = const.tile([1, 512], B16)
        nc.vector.memset(ones512[:], 1.0)
        ones128 = const.tile([1, 128], B16)
        nc.vector.memset(ones128[:], 1.0)

        wpool = ctx.enter_context(tc.tile_pool(name="weights", bufs=1))
        wt = {}
        for nm, dram in [
            ("wqhi", wqhi_d),
            ("wqlo", wqlo_d),
            ("wkhi", wkhi_d),
            ("wklo", wklo_d),
            ("wv", wv_d),
        ]:
            for ei in range(EC):
                t = wpool.tile([128, DHC], B16, tag=f"{nm}{ei}", name=f"{nm}{ei}")
                nc.sync.dma_start(t[:], dram[ei * 128 : (ei + 1) * 128, :])
                wt[nm, ei] = t
        wo_t = []
        for pair in range(2):
            t = wpool.tile([128, e_out], B16, tag=f"wo{pair}", name=f"wo{pair}")
            nc.sync.dma_start(t[:], wo_d[pair * 128 : (pair + 1) * 128, :])
            wo_t.append(t)
        brow = {}
        for nm, dram in [("bq", bq_d), ("bk", bk_d), ("bv", bv_d)]:
            t = wpool.tile([1, DHC], B16, tag=nm, name=f"b{nm}")
            nc.sync.dma_start(t[:], dram[:])
            brow[nm] = t

        # ---- persistent activations ----
        act = ctx.enter_context(tc.tile_pool(name="acts", bufs=1))
        # q/k in pair layout: [128 = headA(64) | headB(64), s], bf16 hi/lo
        qk = {}
        for nm in ("qhi", "qlo", "khi", "klo"):
            for pair in range(2):
                qk[nm, pair] = act.tile([128, s], B16, tag=f"{nm}p{pair}", name=f"{nm}p{pair}")
        v_all = act.tile([128, KT * DHC], B16)  # per k-tile: 4 heads x 64
        outT = [act.tile([128, s], B16, tag=f"outT{p}", name=f"outT{p}") for p in range(2)]

        # ================= Phase 1: projections =================
        with (
            tc.tile_pool(name="xin", bufs=4) as xin,
            tc.tile_pool(name="psproj", bufs=2, space="PSUM") as psproj,
        ):
            for src_hi, src_lo, whi, wlo, bias, dhi, dlo in (
                (qhiT_d, qloT_d, "wqhi", "wqlo", "bq", "qhi", "qlo"),
                (khiT_d, kloT_d, "wkhi", "wklo", "bk", "khi", "klo"),
            ):
                for n in range(NC):
                    xh, xl = [], []
                    for ei in range(EC):
                        th = xin.tile([128, 512], B16, tag="xh", bufs=EC + 2)
                        nc.sync.dma_start(
                            th[:], src_hi[ei * 128 : (ei + 1) * 128, n * 512 : (n + 1) * 512]
                        )
                        xh.append(th)
                        tl = xin.tile([128, 512], B16, tag="xl", bufs=EC + 2)
                        nc.sync.dma_start(
                            tl[:], src_lo[ei * 128 : (ei + 1) * 128, n * 512 : (n + 1) * 512]
                        )
                        xl.append(tl)
                    for pair in range(2):
                        ps = psproj.tile([128, 512], F32)
                        terms = [(whi, xh), (wlo, xh), (whi, xl)]
                        for t_i, (wn, xs) in enumerate(terms):
                            for ei in range(EC):
                                nc.tensor.matmul(
                                    ps[:],
                                    wt[wn, ei][:, pair * 128 : (pair + 1) * 128],
                                    xs[ei][:],
                                    start=(t_i == 0 and ei == 0),
                                    stop=False,
                                )
                        # bias: out[dg, col] += bias[dg] * 1
                        nc.tensor.matmul(
                            ps[:],
                            brow[bias][0:1, pair * 128 : (pair + 1) * 128],
                            ones512[0:1, :],
                            start=False,
                            stop=True,
                        )
                        hi_dst = qk[dhi, pair][:, n * 512 : (n + 1) * 512]
                        lo_dst = qk[dlo, pair][:, n * 512 : (n + 1) * 512]
                        nc.scalar.copy(hi_dst, ps[:])
                        nc.vector.tensor_sub(lo_dst, ps[:], hi_dst)

            # v projection: v[s_chunk, 4 heads * 64] accumulated over e
            with tc.tile_pool(name="psv", bufs=2, space="PSUM") as psv:
                for sc in range(KT):
                    ps = psv.tile([128, DHC], F32)
                    for ei in range(EC):
                        tv = xin.tile([128, 128], B16, tag="xv")
                        nc.sync.dma_start(
                            tv[:], vT_d[ei * 128 : (ei + 1) * 128, sc * 128 : (sc + 1) * 128]
                        )
                        nc.tensor.matmul(
                            ps[:], tv[:], wt["wv", ei][:], start=(ei == 0), stop=False
                        )
                    nc.tensor.matmul(
                        ps[:], ones128[0:1, :], brow["bv"][0:1, :], start=False, stop=True
                    )
                    nc.scalar.copy(v_all[:, sc * DHC : (sc + 1) * DHC], ps[:])

        # ================= Phase 2: attention =================
        with (
            tc.tile_pool(name="amp", bufs=2) as amppool,
            tc.tile_pool(name="pp", bufs=2) as ppool,
            tc.tile_pool(name="pt", bufs=1) as ptpool,
            tc.tile_pool(name="small", bufs=8) as small,
            tc.tile_pool(name="pss", bufs=2, space="PSUM") as pss,
            tc.tile_pool(name="pstr", bufs=1, space="PSUM") as pstr,
            tc.tile_pool(name="pspv", bufs=2, space="PSUM") as pspv,
        ):
            for pair in range(2):
                # p^T accumulator for one q-group (4 q-blocks): [128, kt, 4, 128]
                pts = {}
                for qb in range(QB):
                    qsl = slice(qb * 128, (qb + 1) * 128)
                    if qb % 4 == 0:
                        pts[0] = ptpool.tile([128, KT, 4, 128], B16, tag="ptA", name="ptA")
                        pts[1] = ptpool.tile([128, KT, 4, 128], B16, tag="ptB", name="ptB")
                    amp = [
                        amppool.tile([128, s], F32, tag="ampA", name="ampA"),
                        amppool.tile([128, s], F32, tag="ampB", name="ampB"),
                    ]
                    m = [small.tile([128, 1], F32, tag=f"m{h}", name=f"m{h}") for h in range(2)]
                    for c in range(NC):
                        csl = slice(c * 512, (c + 1) * 512)
                        sps = [pss.tile([128, 512], F32, tag=f"s{h}", name=f"sps{h}") for h in range(2)]
                        terms = (("qhi", "khi"), ("qlo", "khi"), ("qhi", "klo"))
                        for h, (r0, r1, tp) in enumerate(
                            ((0, 64, (0, 0)), (64, 128, (64, 0)))
                        ):
                            for t_i, (qn, kn) in enumerate(terms):
                                nc.tensor.matmul(
                                    sps[h][:],
                                    qk[qn, pair][r0:r1, qsl],
                                    qk[kn, pair][r0:r1, csl],
                                    tile_position=tp,
                                    start=(t_i == 0),
                                    stop=(t_i == 2),
                                )
                        for h in range(2):
                            # amp = |s|, evacuating PSUM promptly (ScalarE)
                            nc.scalar.activation(
                                amp[h][:, csl],
                                sps[h][:],
                                mybir.ActivationFunctionType.Abs,
                            )
                    for h in range(2):
                        nc.vector.tensor_reduce(
                            m[h][:],
                            amp[h][:],
                            axis=mybir.AxisListType.X,
                            op=mybir.AluOpType.max,
                        )
                        negm = small.tile([128, 1], F32, tag=f"negm{h}", name=f"negm{h}")
                        nc.vector.tensor_scalar_mul(negm[:], m[h][:], -1.0)
                        p = ppool.tile([128, s], B16, tag=f"p{h}", name=f"p{h}")
                        sig = small.tile([128, 1], F32, tag=f"sig{h}", name=f"sig{h}")
                        nc.scalar.activation(
                            p[:],
                            amp[h][:],
                            mybir.ActivationFunctionType.Exp,
                            bias=negm[:, 0:1],
                            scale=1.0,
                            accum_out=sig[:, 0:1],
                        )
                        r = small.tile([128, 1], F32, tag=f"r{h}", name=f"r{h}")
                        nc.vector.reciprocal(r[:], sig[:])
                        nc.gpsimd.tensor_scalar_mul(p[:], p[:], r[:, 0:1])
                        pn = p
                        # transpose 128x128 tiles: p^T into PSUM (bf16), TB per bank
                        TB = min(8, KT)
                        for half in range(KT // TB):
                            trp = pstr.tile([128, TB, 128], B16, tag=f"tr{h}", name=f"tr{h}")
                            for j in range(TB):
                                kc = half * TB + j
                                nc.tensor.transpose(
                                    trp[:, j, :],
                                    pn[:, kc * 128 : (kc + 1) * 128],
                                    ident[:],
                                )
                            pt_dst = pts[h][:, half * TB : (half + 1) * TB, qb % 4, :]
                            nc.vector.tensor_copy(pt_dst, trp[:])
                    if qb % 4 == 3:
                        qg = qb // 4
                        pso = pspv.tile([128, 512], F32)
                        for kc in range(KT):
                            for h, tp in ((0, (0, 0)), (1, (0, 64))):
                                hc = pair * 2 + h
                                nc.tensor.matmul(
                                    pso[h * 64 : (h + 1) * 64, :],
                                    v_all[:, kc * DHC + hc * 64 : kc * DHC + (hc + 1) * 64],
                                    pts[h][:, kc, :, :],
                                    tile_position=tp,
                                    start=(kc == 0),
                                    stop=(kc == KT - 1),
                                    skip_group_check=True,
                                )
                        nc.vector.tensor_copy(outT[pair][:, qg * 512 : (qg + 1) * 512], pso[:])

        # ================= Phase 3: out-projection =================
        with (
            tc.tile_pool(name="osb", bufs=2) as osb,
            tc.tile_pool(name="psf", bufs=2, space="PSUM") as psf,
        ):
            for sc in range(KT):
                osl = slice(sc * 128, (sc + 1) * 128)
                ot = osb.tile([128, e_out], F32)
                for e2 in range(E2):
                    ps = psf.tile([128, 512], F32)
                    for pair in range(2):
                        nc.tensor.matmul(
                            ps[:],
                            outT[pair][:, osl],
                            wo_t[pair][:, e2 * 512 : (e2 + 1) * 512],
                            start=(pair == 0),
                            stop=(pair == 1),
                        )
                    nc.vector.tensor_copy(ot[:, e2 * 512 : (e2 + 1) * 512], ps[:])
                nc.sync.dma_start(out_d[osl, :], ot[:])

    nc.compile()
    return nc


_PROG = None


def _get_program():
    global _PROG
    if _PROG is None:
        _PROG = build_program()
    return _PROG


def _split(x):
    hi = x.astype(BF16)
    lo = (x - hi.astype(np.float32)).astype(BF16)
    return hi, lo


def make_in_maps(query, key_, value, Wq, bq, Wk, bk, Wv, bv, Wo):
    per_batch = []
    for b in range(B):
        qT = np.ascontiguousarray(query[b].T.astype(np.float32))
        kT = np.ascontiguousarray(key_[b].T.astype(np.float32))
        vT = np.ascontiguousarray(value[b].T.astype(np.float32)).astype(BF16)
        qhi, qlo = _split(qT)
        khi, klo = _split(kT)
        per_batch.append((qhi, qlo, khi, klo, vT))
    WqT = np.ascontiguousarray((64.0 * Wq).T.astype(np.float32))
    WkT = np.ascontiguousarray(Wk.T.astype(np.float32))
    WvT = np.ascontiguousarray(Wv.T.astype(np.float32))
    WoT = np.ascontiguousarray(Wo.T.astype(np.float32))
    in_maps = []
    for c in range(N_CORES):
        b, g = c // 4, c % 4
        cols = slice(g * DHC, (g + 1) * DHC)
        qhi, qlo, khi, klo, vT = per_batch[b]
        wqhi, wqlo = _split(np.ascontiguousarray(WqT[:, cols]))
        wkhi, wklo = _split(np.ascontiguousarray(WkT[:, cols]))
        in_maps.append(
            {
                "qhiT": qhi,
                "qloT": qlo,
                "khiT": khi,
                "kloT": klo,
                "vT": vT,
                "wqhi": wqhi,
                "wqlo": wqlo,
                "wkhi": wkhi,
                "wklo": wklo,
                "wv": np.ascontiguousarray(WvT[:, cols]).astype(BF16),
                "wo": np.ascontiguousarray(WoT[cols, :]).astype(BF16),
                "bq": (64.0 * bq[cols]).reshape(1, DHC).astype(BF16),
                "bk": bk[cols].reshape(1, DHC).astype(BF16),
                "bv": bv[cols].reshape(1, DHC).astype(BF16),
            }
        )
    return in_maps


TRACE = False


def kernel(query, key_, value, Wq, bq, Wk, bk, Wv, bv, Wo, bo, _res_out=None):
    nc = _get_program()
    in_maps = make_in_maps(query, key_, value, Wq, bq, Wk, bk, Wv, bv, Wo)
    res = run_bass_kernel_spmd(nc, in_maps, list(range(N_CORES)), trace=TRACE)
    out = np.zeros((B, S, E), np.float32)
    for c in range(N_CORES):
        out[c // 4] += res.results[c]["out"]
    out += bo.astype(np.float32)
    if _res_out is not None:
        _res_out.append(res)
    return out
